# revision 1
# baseline (speedup 1.0000x reference)
"""ANI angular symmetry function on 8 TRN2 NeuronCores (Bass/Tile).

out[t, r*4+s] = exp(-ETA*(m-shift_r)^2) * 2*((1+cos(theta-sigma_s))/2)**ZETA
                * fcut(d0)*fcut(d1),  m=(d0+d1)/2, theta=acos(0.95*cos_angle)

Data-parallel over triples T: each core gets T/8 rows (padded).

Engine mapping (only two ACT table sets: natural_log_exp + trig_and_small;
set-bound ACT ops are chained with explicit deps so the scheduler cannot
interleave sets and thrash ACT_TABLE_LOADs):
  dot   = sum(v0*v1)                               DVE mult + reduce
  t     = 0.95*dot/sqrt((d0*d1)^2-(0.95*dot)^2)    Square/Ln/Exp
  theta = pi/2 - atan(t);  cos(theta-sigma_s) = sin(atan(t)+sigma_s)
  sections 2,3 via sin(pi-x) identity (scale=-1) keep Sin args in [-1,2.5]
  ((1+c)/2)^ZETA = exp(ZETA*ln(0.5*c+0.5))         flat Ln/Exp over (F_S,4)
  radial: exp(-Square(sqrt(ETA)/2*msum - sqrt(ETA)*s_r) + ln(0.5)) -> bf16
  outer product: one DVE tensor_tensor per sub-chunk with 4-dim APs
"""
import math
import os
import numpy as np

ETA = 12.5
ZETA = 14.1
CUTOFF = 3.5
NCORES = 8
P = 128

OUT_BF16 = os.environ.get("ANI_OUT_BF16", "1") == "1"
NO_CHAIN = os.environ.get("ANI_NO_CHAIN", "0") == "1"
R_F32 = os.environ.get("ANI_R_F32", "0") == "1"
SMAJ = os.environ.get("ANI_SMAJ", "0") == "1"

_cache = {}
last_results = None  # BassKernelResults from the most recent run (for test.py)


def _geometry(T):
    per = (T + NCORES - 1) // NCORES
    # f_tot multiple of 72: NSUP=3 supers, F1=F_S/4, F3=F_S/8 all integral
    f_tot = ((per + P * 72 - 1) // (P * 72)) * 72
    return per, f_tot


def _build(shifts, sections, f_tot):
    import concourse.bass as bass
    import concourse.bacc as bacc
    import concourse.tile as tile
    from concourse import mybir
    from concourse.tile_rust import add_dep_helper

    AF = mybir.ActivationFunctionType
    ALU = mybir.AluOpType
    F32 = mybir.dt.float32
    BF16 = mybir.dt.bfloat16
    OUT_DT = BF16 if OUT_BF16 else F32

    NSUP = 3
    sup_sizes = [f_tot // NSUP] * NSUP
    assert sum(sup_sizes) == f_tot and all(x % 24 == 0 for x in sup_sizes)
    npad = P * f_tot

    SQS = math.sqrt(ETA) / 2.0
    shift_bias = [-math.sqrt(ETA) * float(s) for s in shifts]
    sig = [float(x) for x in sections]

    # Force ACT table-set selection to the two sets that cover all our
    # functions; the rust pass greedily picks the first set containing a
    # func (ln->natural_log, exp->exp_and_others) which doubles the loads.
    import concourse.bacc as _bacc_mod
    from concourse.hw_specs import get_activation_tables as _real_tabs
    _KEEP = {"natural_log_exp_and_others", "trig_and_small"}

    def _filtered_tabs(arch):
        real = _real_tabs(arch)
        if not _KEEP.issubset(real.keys()):
            return real  # unexpected act_info layout: don't filter
        return {name: (fns if name in _KEEP else set())
                for name, fns in real.items()}

    _bacc_mod.get_activation_tables = _filtered_tabs

    nc = bacc.Bacc("TRN2", target_bir_lowering=False, debug=False,
                   num_devices=NCORES)

    d_dram = nc.dram_tensor("d", [2, npad], F32, kind="ExternalInput")
    v_dram = nc.dram_tensor("v", [2, npad, 3], F32, kind="ExternalInput")
    o_dram = nc.dram_tensor("out", [npad, 32], OUT_DT, kind="ExternalOutput")

    d_v = [d_dram[i].rearrange("(p f) -> p f", p=P) for i in range(2)]
    v_v = [v_dram[i].rearrange("(p f) c -> p (f c)", p=P) for i in range(2)]
    o_v = o_dram.rearrange("(p f) k -> p (f k)", p=P)

    # chain all table-set-bound ACT ops in program order
    prev_act = [None]
    SET_BOUND = {AF.Ln, AF.Exp, AF.Sin, AF.Arctan}

    def act(out, in_, func, **kw):
        ins = nc.scalar.activation(out, in_, func, **kw)
        if func in SET_BOUND and not NO_CHAIN:
            if prev_act[0] is not None:
                add_dep_helper(ins.ins, prev_act[0].ins, sync=False,
                               reason="act-set-order")
            prev_act[0] = ins
        return ins

    with tile.TileContext(nc) as tc:
        import contextlib
        ctx = contextlib.ExitStack()
        with ctx:
            consts = ctx.enter_context(tc.tile_pool(name="consts", bufs=1))
            pers2 = ctx.enter_context(tc.tile_pool(name="pers2", bufs=2))
            rpool = ctx.enter_context(tc.tile_pool(name="rpool", bufs=1))
            p1 = ctx.enter_context(tc.tile_pool(name="p1", bufs=2))
            op = ctx.enter_context(tc.tile_pool(name="op", bufs=3))

            cvals = [math.log(0.95), math.log(0.5), 0.5, math.pi / 2,
                     sig[0], sig[1], math.pi - sig[2], math.pi - sig[3]]
            cvals += shift_bias
            cb = consts.tile([P, len(cvals)], F32, name="cb")
            for i, v in enumerate(cvals):
                nc.vector.memset(cb[:, i:i + 1], v)
            B_LN095, B_LN05, B_HALF, B_PI2 = (cb[:, i:i + 1] for i in range(4))
            B_SIG = [cb[:, 4 + i:5 + i] for i in range(4)]
            B_SH = [cb[:, 8 + i:9 + i] for i in range(8)]

            def emit_p1(sc, j_sup, F_S, tiles):
                """pass 1 [ln/exp]: dot, t = cot(theta), msum for super sc."""
                F1 = F_S // 4
                N1 = 4
                d_sb = pers2.tile([P, 2 * F_S], F32, name="d_sb", tag="d_sb")
                nc.sync.dma_start(d_sb[:, :F_S], d_v[0][:, j_sup:j_sup + F_S])
                nc.sync.dma_start(d_sb[:, F_S:], d_v[1][:, j_sup:j_sup + F_S])
                t_sb = pers2.tile([P, F_S], F32, name="t_sb", tag="t_sb")
                msum = pers2.tile([P, F_S], F32, name="msum", tag="msum")
                for i1 in range(N1):
                    a, b = i1 * F1, (i1 + 1) * F1
                    d0s, d1s = d_sb[:, a:b], d_sb[:, F_S + a:F_S + b]
                    v0t = p1.tile([P, 3 * F1], F32, name="v0t", tag="v0t")
                    v1t = p1.tile([P, 3 * F1], F32, name="v1t", tag="v1t")
                    nc.sync.dma_start(
                        v0t[:], v_v[0][:, 3 * (j_sup + a):3 * (j_sup + b)])
                    nc.sync.dma_start(
                        v1t[:], v_v[1][:, 3 * (j_sup + a):3 * (j_sup + b)])
                    nc.vector.tensor_tensor(v0t[:], v0t[:], v1t[:], ALU.mult)
                    dott = p1.tile([P, F1], F32, name="dott", tag="dott")
                    nc.vector.tensor_reduce(
                        dott[:], v0t[:].rearrange("p (f c) -> p f c", c=3),
                        axis=mybir.AxisListType.X, op=ALU.add)
                    q1 = p1.tile([P, F1], F32, name="q1", tag="q1")
                    q2 = p1.tile([P, F1], F32, name="q2", tag="q2")
                    nc.vector.tensor_tensor(q1[:], d0s, d1s, ALU.mult)
                    nc.scalar.activation(q1[:], q1[:], AF.Square)
                    nc.scalar.activation(q2[:], dott[:], AF.Square, scale=0.95)
                    nc.vector.tensor_tensor(q1[:], q1[:], q2[:], ALU.subtract)
                    act(q1[:], q1[:], AF.Ln)
                    act(q1[:], q1[:], AF.Exp, bias=B_LN095, scale=-0.5)
                    nc.vector.tensor_tensor(t_sb[:, a:b], dott[:], q1[:],
                                            ALU.mult)
                    nc.vector.tensor_tensor(msum[:, a:b], d0s, d1s, ALU.add)
                tiles[sc] = (d_sb, t_sb, msum)

            def emit_mid(sc, F_S, tiles):
                """trig + LHP + radial for super sc; returns (sins, r_full)."""
                FQ = F_S // 4
                NQ = 4
                d_sb, t_sb, msum = tiles[sc]
                sins = (pers2.tile([P, 4, F_S], F32, name="sins", tag="sins")
                        if SMAJ else
                        pers2.tile([P, F_S, 4], F32, name="sins", tag="sins"))
                def sview(i):
                    return sins[:, i, :] if SMAJ else sins[:, :, i]
                fcq = pers2.tile([P, F_S], F32, name="fcq", tag="fcq")
                fch = rpool.tile([P, F_S], F32, name="fch", tag="fch")
                r_full = rpool.tile([P, 8, F_S], F32 if R_F32 else BF16,
                                    name="r_full", tag="r_full")
                # trig set
                act(t_sb[:], t_sb[:], AF.Arctan)
                act(sview(0), t_sb[:], AF.Sin, bias=B_SIG[0], scale=1.0)
                act(sview(1), t_sb[:], AF.Sin, bias=B_SIG[1], scale=1.0)
                act(sview(2), t_sb[:], AF.Sin, bias=B_SIG[2], scale=-1.0)
                act(sview(3), t_sb[:], AF.Sin, bias=B_SIG[3], scale=-1.0)
                act(fch[:], d_sb[:, F_S:], AF.Sin,
                    bias=B_PI2, scale=-math.pi / CUTOFF)
                nc.vector.tensor_scalar_add(fcq[:], fch[:], 1.0)
                act(fch[:], d_sb[:, :F_S], AF.Sin,
                    bias=B_PI2, scale=-math.pi / CUTOFF)
                nc.vector.scalar_tensor_tensor(
                    fcq[:], fch[:], 1.0, fcq[:], ALU.add, ALU.mult)
                # ln/exp set: H = (0.5*sins+0.5)^ZETA then P = H*fcq,
                # quarter-split so the first outer ops unblock earlier
                FH = F_S // 4
                for hh in range(4):
                    ha, hb = hh * FH, (hh + 1) * FH
                    sq = sins[:, :, ha:hb] if SMAJ else sins[:, ha:hb, :]
                    sfl = sq
                    act(sfl, sfl, AF.Ln, bias=B_HALF, scale=0.5)
                    act(sfl, sfl, AF.Exp, scale=ZETA)
                    fb = fcq[:, ha:hb]
                    fcq_b = bass.AP(tensor=fb.tensor, offset=fb.offset,
                                    ap=([fb.ap[0], [0, 4], [1, FH]] if SMAJ
                                        else [fb.ap[0], [1, FH], [0, 4]]))
                    nc.vector.tensor_tensor(sq, sq, fcq_b, ALU.mult)
                # radial: R[r, j] = exp(-(SQS*m - sqrt(ETA)*s_r)^2 + ln(0.5))
                for q in range(NQ):
                    a, b = q * FQ, (q + 1) * FQ
                    rq = rpool.tile([P, 8, FQ], F32, name="rq", tag="rq")
                    for r in range(8):
                        nc.scalar.activation(rq[:, r, :], msum[:, a:b],
                                             AF.Square, bias=B_SH[r],
                                             scale=SQS)
                    act(r_full[:, :, a:b], rq[:], AF.Exp,
                        bias=B_LN05, scale=-1.0)
                return sins, r_full

            def emit_outer(sc, j_sup, F_S, sins, r_full):
                """pass 3: pure DVE outer product + DMA out."""
                F3 = F_S // 8
                N3 = 8
                rf = r_full[:].rearrange("p r f -> p (r f)")
                pf = (sins[:].rearrange("p s f -> p (s f)") if SMAJ
                      else sins[:].rearrange("p f s -> p (f s)"))
                for i3 in range(N3):
                    a = i3 * F3
                    ot = op.tile([P, F3 * 32], OUT_DT, name="ot", tag="ot")
                    ob = ot[:]
                    o_ap = bass.AP(tensor=ob.tensor, offset=ob.offset,
                                   ap=[ob.ap[0], [32, F3], [4, 8], [1, 4]])
                    r_ap = bass.AP(tensor=rf.tensor, offset=rf.offset + a,
                                   ap=[rf.ap[0], [1, F3], [F_S, 8], [0, 4]])
                    p_ap = bass.AP(
                        tensor=pf.tensor,
                        offset=pf.offset + (a if SMAJ else 4 * a),
                        ap=([pf.ap[0], [1, F3], [0, 8], [F_S, 4]] if SMAJ
                            else [pf.ap[0], [4, F3], [0, 8], [1, 4]]))
                    nc.vector.tensor_tensor(o_ap, r_ap, p_ap, ALU.mult)
                    g0 = 32 * (j_sup + a)
                    # issue output stores via idle GPSIMD (SWDGE) so they
                    # don't serialize behind input-load issue on the sync
                    # sequencer; DVE here is 1-port TT ops, so no SWDGE
                    # port starvation
                    odma = (nc.sync if os.environ.get("ANI_ODMA") == "s"
                            else nc.gpsimd)
                    odma.dma_start(o_v[:, g0:g0 + 32 * F3], ot[:])

            tiles = {}
            j_sup = 0
            for sc, fs in enumerate(sup_sizes):
                emit_p1(sc, j_sup, fs, tiles)
                sins, r_full = emit_mid(sc, fs, tiles)
                emit_outer(sc, j_sup, fs, sins, r_full)
                j_sup += fs

    nc.compile()
    return nc


def kernel(tri_distances, tri_vectors, shifts, sections):
    from concourse.bass_utils import run_bass_kernel_spmd

    T = tri_distances.shape[1]
    per, f_tot = _geometry(T)
    npad = P * f_tot

    key = (T, OUT_BF16, NO_CHAIN, R_F32, os.environ.get('ANI_ODMA'), SMAJ)
    if key not in _cache:
        _cache[key] = _build(np.asarray(shifts, np.float64),
                             np.asarray(sections, np.float64), f_tot)
    nc = _cache[key]

    d_full = np.ascontiguousarray(np.asarray(tri_distances, np.float32))
    v_full = np.ascontiguousarray(np.asarray(tri_vectors, np.float32))

    in_maps = []
    for i in range(NCORES):
        lo = i * per
        hi = min(lo + per, T)
        n = hi - lo
        dpad = np.empty((2, npad), np.float32)
        dpad[:, :n] = d_full[:, lo:hi]
        dpad[:, n:] = 1.0
        vpad = np.empty((2, npad, 3), np.float32)
        vpad[:, :n] = v_full[:, lo:hi]
        vpad[:, n:, 0] = 1.0
        vpad[:, n:, 1:] = 0.0
        in_maps.append({"d": dpad, "v": vpad})

    trace = os.environ.get("ANI_TRACE", "0") == "1"
    res = None
    last_err = None
    for _attempt in range(3):
        try:
            res = run_bass_kernel_spmd(nc, in_maps, list(range(NCORES)),
                                       trace=trace)
            break
        except Exception as e:  # rare transient device errors; retry
            last_err = e
    if res is None:
        raise last_err
    global last_results
    last_results = res
    parts = []
    for i in range(NCORES):
        lo = i * per
        n = min(lo + per, T) - lo
        o = res.results[i]["out"][:n]
        parts.append(np.asarray(o, dtype=np.float32))
    return np.concatenate(parts, axis=0)

